# revision 15
# baseline (speedup 1.0000x reference)
"""Trainium2 Bass kernel for LoRA-augmented causal attention.

Reference computation (per nn_Attention_31688268710508):
  x:(B,S,D) -> q/k/v = x@W* + broadcast LoRA + shared head-offset LoRA,
  RoPE(q,k), causal softmax attention per (b,head), out-proj with wo.

Strategy (8 NeuronCores, tensor-parallel over heads):
  * All rank-8 LoRA terms are folded into effective projection weights on
    the host (x @ (A@B) == (x@A)@B up to fp rounding).  The softmax
    1/sqrt(HD) scale is folded into Wq.
  * RoPE pairs (2i,2i+1) are moved to (i, i+64) by permuting Wq/Wk columns
    (scores are invariant to a shared q/k head-dim permutation), making
    RoPE a half-partition-block rotation in the on-chip [hd, s] layout.
  * fp16 operands everywhere on the PE (fp32 PSUM accumulation).
  * Projections (W stationary, x^T streamed) and causal attention
    (S^T = K^T.T Q^T blocks -> exp -> fp16 P^T -> V-stationary O^T
    matmuls; softmax denominator via DVE-accumulated P^T + one ones-matmul)
    are emitted INTERLEAVED so projection matmuls for (b,h) segment k+1
    hide the exp latency of attention segment k and keep the PE dense/warm.
  * Per-head AllToAll redistributes O^T by sequence rows; the first
    exchange overlaps the second head's attention.  Each core then
    multiplies with full wo for its 512-row output shard.
  * DMA channels: qSP (sync) streams x^T slabs in per-db chunks; qAct
    (scalar) carries weight panels / tables / wo / a2a-out / out_rows;
    gpsimd SWDGE carries compute-dependent a2a-in stores so they never
    head-of-line-block a load queue.  Weight panels and wo are pre-laid
    out on the host as [128, nblk, 128] so their DMAs are contiguous.
  * wo's h=0 half is preloaded into an outer SBUF tile during attention
    segment 3 so the out-projection starts the moment AllToAll(h=0)
    lands; the h=1 half streams in under the h=0 matmuls.
"""

import math
import os
import sys
from contextlib import ExitStack

import numpy as np

for _p in ("/opt/trn_rl_repo", "/root/.axon_site/_ro/trn_rl_repo"):
    if os.path.isdir(_p) and _p not in sys.path:
        sys.path.insert(0, _p)

import concourse.bass as bass  # noqa: E402
import concourse.mybir as mybir  # noqa: E402
import concourse.tile as tile  # noqa: E402
from concourse import bacc  # noqa: E402
from concourse.masks import make_identity  # noqa: E402

F32 = mybir.dt.float32
F16 = mybir.dt.float16
EXP = mybir.ActivationFunctionType.Exp

NEG = -1.0e30


class Cfg:
    def __init__(self, B=2, S=2048, D=2048, HEADS=16, NCORES=8):
        self.B, self.S, self.D, self.NCORES = B, S, D, NCORES
        self.HD = 128
        self.HPC = HEADS // NCORES          # heads per core
        self.BS = B * S
        self.RPC = self.BS // NCORES        # output rows per core
        self.ST = 512                       # free-dim tile (q tile, s tile)
        self.SG = min(512, self.S)          # x^T slab width (s cols)
        self.NDB = D // 128                 # contraction blocks
        self.GPB = self.S // self.SG        # slabs per batch b
        self.NST_G = self.SG // self.ST     # s tiles per slab
        self.KPG = 2                        # k-blocks per exp group
        self.NHB = (HEADS * self.HD) // 128  # out-proj contraction blocks
        assert self.HD == 128 and D % 512 == 0 and S % self.ST == 0
        assert self.RPC % 128 == 0 and self.SG % self.ST == 0


def build_bass(cfg: Cfg) -> bass.Bass:
    B, S, D = cfg.B, cfg.S, cfg.D
    HPC, HD, ST, SG = cfg.HPC, cfg.HD, cfg.ST, cfg.SG
    NC_, RPC = cfg.NCORES, cfg.RPC

    nc = bacc.Bacc("TRN2", target_bir_lowering=False, debug=False,
                   num_devices=NC_)

    xT = nc.declare_dram_parameter("xT", [D, cfg.BS], F16, isOutput=False)
    # weight panels pre-laid out host-side: [HPC, 128, NDB, 128] per proj
    wq = nc.declare_dram_parameter("wq", [HPC, 128, cfg.NDB, 128], F16,
                                   isOutput=False)
    wk = nc.declare_dram_parameter("wk", [HPC, 128, cfg.NDB, 128], F16,
                                   isOutput=False)
    wv = nc.declare_dram_parameter("wv", [HPC, 128, cfg.NDB, 128], F16,
                                   isOutput=False)
    # wo pre-permuted h-major: [HPC, 128, NHB//HPC, D]
    wo = nc.declare_dram_parameter("wo", [HPC, 128, cfg.NHB // HPC, D], F16,
                                   isOutput=False)
    cosT = nc.declare_dram_parameter("cosT", [128, S], F16, isOutput=False)
    sinT = nc.declare_dram_parameter("sinT", [128, S], F16, isOutput=False)
    band = nc.declare_dram_parameter("band", [128, 128], F32,
                                     isOutput=False)
    out_rows = nc.declare_dram_parameter("out_rows", [RPC, D], F32,
                                         isOutput=True)

    a2a_ins = [nc.dram_tensor(f"a2a_in{h}", [NC_, HD, RPC], F16)
               for h in range(HPC)]
    a2a_outs = [nc.dram_tensor(f"a2a_out{h}", [NC_, HD, RPC], F16)
                for h in range(HPC)]

    with tile.TileContext(nc) as tc:
        with ExitStack() as ctx:
            constp = ctx.enter_context(tc.tile_pool(name="const", bufs=1))
            ident = constp.tile([128, 128], F16)
            make_identity(nc, ident)
            band_sb = constp.tile([128, 128], F32)
            ones_sb = constp.tile([128, 128], F16)
            nc.vector.memset(ones_sb, 1.0)

            qtp = ctx.enter_context(tc.tile_pool(name="qt", bufs=1))
            ktp = ctx.enter_context(tc.tile_pool(name="kt", bufs=1))
            vhp = ctx.enter_context(tc.tile_pool(name="vhat", bufs=1))
            wosp = ctx.enter_context(tc.tile_pool(name="wo_sb", bufs=1))
            QT = qtp.tile([128, HPC, cfg.BS], F16)
            KT = ktp.tile([128, HPC, cfg.BS], F16)
            Vhat = vhp.tile([128, HPC, B, S // 128, 128], F16)
            WO = wosp.tile([128, cfg.NHB // HPC, D], F16)

            st = _State(nc, tc, cfg, xT, (wq, wk, wv), wo, WO, cosT, sinT,
                        band, band_sb, ones_sb, ident, QT, KT, Vhat,
                        a2a_ins, a2a_outs)
            with ExitStack() as phase_ctx:
                st.open_phase_pools(phase_ctx)

                segs = [(b, h) for b in range(B) for h in range(HPC)]
                # lag-1 interleave: attention segment k runs against
                # projection segment k+1's matmuls.
                pgens = [st.proj_segment(b, h) for (b, h) in segs]
                agens = [st.attn_segment(b, h) for (b, h) in segs]
                for _ in pgens[0]:
                    pass
                for k, ag in enumerate(agens):
                    # preload wo's h=0 half while attention still runs,
                    # one 512KB sub-chunk per yield so weight panels are
                    # not delayed in the qAct FIFO
                    wo_feed = (iter(range(cfg.NHB // HPC))
                               if k == len(segs) - 2 else iter(()))
                    pg = pgens[k + 1] if k + 1 < len(segs) else None
                    for _ in ag:
                        if pg is not None:
                            next(pg, None)
                        j = next(wo_feed, None)
                        if j is not None:
                            nc.sync.dma_start(out=st.WO[:, j, :],
                                              in_=st.wo_dram[0, :, j, :])
                    if pg is not None:
                        for _ in pg:
                            pass
                    b, h = segs[k]
                    if b == B - 1:
                        nc.gpsimd.collective_compute(
                            "AllToAll",
                            mybir.AluOpType.bypass,
                            replica_groups=[list(range(NC_))],
                            ins=[a2a_ins[h][:, :, :]],
                            outs=[a2a_outs[h][:, :, :]],
                        )

            with (
                tc.tile_pool(name="aot_sb", bufs=1) as aip,
                tc.tile_pool(name="accp", bufs=1) as accp,
                tc.tile_pool(name="ob", bufs=4) as obp,
                tc.tile_pool(name="psU", bufs=8, space="PSUM") as psU,
            ):
                _outproj(nc, cfg, st, a2a_outs, aip, accp, obp, psU,
                         out_rows)

    nc.finalize()
    return nc


class _State:
    """Shared emission state for the interleaved proj/attention phases."""

    def __init__(self, nc, tc, cfg, xT, w_drams, wo_dram, WO, cosT, sinT,
                 band, band_sb, ones_sb, ident, QT, KT, Vhat, a2a_ins,
                 a2a_outs):
        self.nc, self.tc, self.cfg = nc, tc, cfg
        self.xT, self.w_drams = xT, w_drams
        self.wo_dram, self.WO = wo_dram, WO
        self.cosT, self.sinT, self.band = cosT, sinT, band
        self.band_sb, self.ones_sb, self.ident = band_sb, ones_sb, ident
        self.QT, self.KT, self.Vhat = QT, KT, Vhat
        self.a2a_ins, self.a2a_outs = a2a_ins, a2a_outs
        self.slabs = {}          # g -> slab tile
        self.tables_loaded = False

    def open_phase_pools(self, ctx):
        tc = self.tc
        self.xp = ctx.enter_context(tc.tile_pool(name="xslab", bufs=4))
        self.wp = ctx.enter_context(tc.tile_pool(name="wres", bufs=1))
        self.wpanels = {}
        self.tbp = ctx.enter_context(tc.tile_pool(name="tables", bufs=1))
        self.rp = ctx.enter_context(tc.tile_pool(name="ropet", bufs=3))
        self.stp = ctx.enter_context(tc.tile_pool(name="vstage", bufs=3))
        self.ptp = ctx.enter_context(tc.tile_pool(name="ptile", bufs=3))
        self.pap = ctx.enter_context(tc.tile_pool(name="pacc", bufs=2))
        self.aotp = ctx.enter_context(tc.tile_pool(name="aot", bufs=3))
        self.rcp = ctx.enter_context(tc.tile_pool(name="rec", bufs=2))
        self.psS = ctx.enter_context(
            tc.tile_pool(name="psS", bufs=4, space="PSUM"))
        self.psD = ctx.enter_context(
            tc.tile_pool(name="psD", bufs=1, space="PSUM"))
        self.psJ = ctx.enter_context(
            tc.tile_pool(name="psJ", bufs=2, space="PSUM"))
        self.psO = ctx.enter_context(
            tc.tile_pool(name="psO", bufs=1, space="PSUM"))
        self.cos_sb = self.tbp.tile([128, self.cfg.S], F16)
        self.sin_sb = self.tbp.tile([128, self.cfg.S], F16)

    def load_wo_chunk(self, h):
        """Stream wo's half for head h into the resident WO tile (qAct)."""
        nc, cfg = self.nc, self.cfg
        nh = cfg.NHB // cfg.HPC
        for j in range(nh):
            nc.sync.dma_start(out=self.WO[:, j, :],
                              in_=self.wo_dram[h, :, j, :])

    def _wpanel(self, proj, h):
        """Resident weight panel [128, NDB, 128], loaded once (qAct)."""
        key = (proj, h)
        if key not in self.wpanels:
            t = self.wp.tile([128, self.cfg.NDB, 128], F16,
                             tag=f"w{proj}_{h}", name=f"w{proj}_{h}")
            nc = self.nc
            for c in range(0, self.cfg.NDB, 4):
                nc.scalar.dma_start(out=t[:, c:c + 4, :],
                                    in_=self.w_drams[proj][h, :, c:c + 4, :])
            self.wpanels[key] = t
        return self.wpanels[key]

    def _slab(self, g):
        """Whole-slab x^T tile [128, NDB, SG]; chunked DMAs (qSP)."""
        nc, cfg = self.nc, self.cfg
        if g not in self.slabs:
            t = self.xp.tile([128, cfg.NDB, cfg.SG], F16, tag="xs",
                             name=f"xs{g}")
            src = self.xT[:, g * cfg.SG:(g + 1) * cfg.SG].rearrange(
                "(db p) c -> p db c", p=128)
            if g == 0:
                # fine chunks so the first matmuls start after ~128KB
                for db in range(4):
                    nc.sync.dma_start(out=t[:, db, :], in_=src[:, db, :])
                for c in range(1, 4):
                    nc.sync.dma_start(out=t[:, 4 * c:4 * (c + 1), :],
                                      in_=src[:, 4 * c:4 * (c + 1), :])
            else:
                for c in range(4):
                    nc.sync.dma_start(out=t[:, 4 * c:4 * (c + 1), :],
                                      in_=src[:, 4 * c:4 * (c + 1), :])
            self.slabs[g] = t
            if not self.tables_loaded:
                self.tables_loaded = True
                nc.sync.dma_start(out=self.band_sb, in_=self.band[:, :])
                nc.sync.dma_start(out=self.cos_sb, in_=self.cosT[:, :])
                nc.sync.dma_start(out=self.sin_sb, in_=self.sinT[:, :])
        return self.slabs[g]

    def proj_segment(self, b, h):
        """Generator: projections (Q,K,V) of head h over batch b's slabs.

        Yields after each contraction step (NST_G matmuls) so the driver
        can interleave attention work.
        """
        nc, cfg = self.nc, self.cfg
        S, SG, ST, NDB = cfg.S, cfg.SG, cfg.ST, cfg.NDB
        for g in range(b * cfg.GPB, (b + 1) * cfg.GPB):
            xs = self._slab(g)
            for proj in range(3):
                w_t = self._wpanel(proj, h)
                accs = [self.psJ.tile([128, ST], F32, tag="pj",
                                      name=f"pj{st_i}")
                        for st_i in range(cfg.NST_G)]
                for db in range(NDB):
                    for st_i in range(cfg.NST_G):
                        nc.tensor.matmul(
                            accs[st_i],
                            lhsT=w_t[:, db, :],
                            rhs=xs[:, db, st_i * ST:(st_i + 1) * ST],
                            start=(db == 0), stop=(db == NDB - 1))
                    yield
                for st_i in range(cfg.NST_G):
                    gcol = g * SG + st_i * ST
                    scol = gcol % S
                    acc = accs[st_i]
                    if proj < 2:
                        dst = (self.QT if proj == 0 else
                               self.KT)[:, h, gcol:gcol + ST]
                        t1 = self.rp.tile([128, ST], F32, tag="t1")
                        nc.vector.tensor_mul(
                            t1, acc, self.cos_sb[:, scol:scol + ST])
                        t2 = self.rp.tile([128, ST], F32, tag="t2")
                        nc.vector.tensor_mul(
                            t2[0:64], acc[64:128],
                            self.sin_sb[0:64, scol:scol + ST])
                        nc.vector.tensor_mul(
                            t2[64:128], acc[0:64],
                            self.sin_sb[64:128, scol:scol + ST])
                        nc.vector.tensor_sub(dst[0:64], t1[0:64], t2[0:64])
                        nc.vector.tensor_add(dst[64:128], t1[64:128],
                                             t2[64:128])
                    else:
                        vstage = self.stp.tile([128, ST], F16, tag="vst")
                        nc.scalar.copy(out=vstage, in_=acc)
                        for jj in range(ST // 128):
                            gc = gcol + jj * 128
                            b_idx, kblk = gc // S, (gc % S) // 128
                            nc.scalar.dma_start(
                                out=self.Vhat[:, h, b_idx, kblk, :],
                                in_=vstage[:, jj * 128:(jj + 1) * 128],
                                transpose=True)
                    yield

    def attn_segment(self, b, h):
        """Generator: causal attention for (h, b).

        KPG-free pipeline: scores for k-block i+2 are emitted before the
        PV of block i (lookahead 2) so the exp latency hides under queued
        matmuls.  Diagonal blocks compute only columns [d, ST) with a
        fixed 128-wide triangle mask; the excluded pt region is zeroed
        off the critical chain.
        """
        nc, cfg = self.nc, self.cfg
        S, ST, RPC = cfg.S, cfg.ST, cfg.RPC
        LA = 3
        tri = self.band_sb
        for qt in range(S // ST):
            q0 = qt * ST
            nkb = (q0 + ST) // 128
            po = self.psO.tile([128, ST], F32, tag="po")
            pa = self.pap.tile([128, ST], F16, tag="pa")
            state = {}
            pa_state = {}

            def emit_sc(kb):
                d = kb * 128 - q0
                lo = max(d, 0)
                ps = self.psS.tile([128, ST], F32, tag="ps")
                pt = self.ptp.tile([128, ST], F16, tag="pt")
                nc.tensor.matmul(
                    ps[:, lo:ST],
                    lhsT=self.KT[:, h, b * S + kb * 128:
                                 b * S + kb * 128 + 128],
                    rhs=self.QT[:, h, b * S + q0 + lo:b * S + q0 + ST],
                    start=True, stop=True)
                if lo > 0:
                    nc.vector.memset(pt[:, 0:lo], 0.0)
                if d >= 0:
                    nc.vector.tensor_add(ps[:, lo:lo + 128],
                                         ps[:, lo:lo + 128], tri)
                nc.scalar.activation(pt[:, lo:ST], ps[:, lo:ST], EXP)
                if kb > 0:
                    flush_pa(kb - 1)
                state[kb] = (pt, lo)
                pa_state[kb] = (pt, lo)

            def flush_pa(kb):
                pt, lo = pa_state.pop(kb)
                if kb == 0:
                    nc.vector.tensor_copy(pa, pt)
                else:
                    nc.vector.tensor_add(pa[:, lo:ST], pa[:, lo:ST],
                                         pt[:, lo:ST])

            def emit_pv(kb):
                pt, lo = state.pop(kb)
                nc.tensor.matmul(
                    po[:, lo:ST] if lo > 0 else po,
                    lhsT=self.Vhat[:, h, b, kb, :],
                    rhs=pt[:, lo:ST],
                    start=(kb == 0), stop=(kb == nkb - 1))

            for i in range(nkb + LA):
                if i < nkb:
                    emit_sc(i)
                if i >= LA:
                    emit_pv(i - LA)
                if i % 2 == 1 or i >= nkb:
                    yield
            flush_pa(nkb - 1)
            pd = self.psD.tile([128, ST], F32, tag="pd")
            nc.tensor.matmul(pd, lhsT=self.ones_sb, rhs=pa,
                             start=True, stop=True)
            rec = self.rcp.tile([128, ST], F32, tag="rec")
            nc.vector.reciprocal_approx_fast(out=rec, in_=pd)
            aot = self.aotp.tile([128, ST], F16, tag="aot")
            nc.vector.tensor_mul(aot, po, rec)
            CSZ = min(ST, RPC)
            for ci in range(ST // CSZ):
                r0 = b * S + q0 + ci * CSZ
                nc.sync.dma_start(
                    out=self.a2a_ins[h][r0 // RPC, :,
                                        r0 % RPC:r0 % RPC + CSZ],
                    in_=aot[:, ci * CSZ:(ci + 1) * CSZ])


def _outproj(nc, cfg, st, a2a_outs, aip, accp, obp, psU, out_rows):
    """out_rows[s', :] = AO^T_full[:, s'].T @ wo, contraction over heads.

    One PSUM round per (h, ss): round h accumulates that head's 8 blocks.
    The h=0 half of wo is already resident (preloaded during attention);
    the h=1 half streams in under the h=0 matmuls.  h=0 partial sums park
    in SBUF and are added during the h=1 pass.
    """
    D, HPC, RPC = cfg.D, cfg.HPC, cfg.RPC
    NH = cfg.NHB // HPC                   # blocks per head-group (8)
    NDC = D // 512
    NSS = RPC // 128

    # a2a_out tiles; h>0 loads emitted late (with that h's wo reload) so
    # the qSP FIFO order is [aip h0][wo h1][aip h1][out_rows]
    aot_sb = {}

    def load_aip(h):
        for j in range(NH):
            a = aip.tile([128, RPC], F16, tag=f"ai{h}_{j}",
                         name=f"ai_sb{h}_{j}")
            nc.sync.dma_start(out=a, in_=a2a_outs[h][j, :, :])
            aot_sb[(h, j)] = a

    load_aip(0)
    def wo_rhs(h, j, c0, c1):
        if h == 0:
            return st.WO[:, j, c0:c1]
        # h=1 wo blocks live in the dead QT/KT space: panels 0-3 in QT,
        # 4-7 in KT, flat-addressed [hh, 2048-col half]
        tile_ = st.QT if j < 4 else st.KT
        base = ((j % 4) % 2) * 2048
        return tile_[:, (j % 4) // 2, base + c0:base + c1]

    acc = {}
    for h in range(HPC):
        if h > 0:
            # emitted AFTER attention (QT/KT last reads) so the WAR frees
            # immediately; these loads hide under h=0's matmuls
            for j in range(NH):
                nc.sync.dma_start(out=wo_rhs(h, j, 0, D),
                                  in_=st.wo_dram[h, :, j, :])
            load_aip(h)
        for ss in range(NSS):
            pu = [psU.tile([128, 512], F32, tag="pu", name=f"pu_{dct}")
                  for dct in range(NDC)]
            for j in range(NH):
                for dct in range(NDC):
                    nc.tensor.matmul(
                        pu[dct],
                        lhsT=aot_sb[(h, j)][:, ss * 128:(ss + 1) * 128],
                        rhs=wo_rhs(h, j, dct * 512, (dct + 1) * 512),
                        start=(j == 0), stop=(j == NH - 1))
            for dct in range(NDC):
                if h == 0 and HPC > 1:
                    t = accp.tile([128, 512], F32, tag=f"acc{ss}_{dct}",
                                  name=f"acc{ss}_{dct}")
                    nc.vector.tensor_copy(t, pu[dct])
                    acc[(ss, dct)] = t
                else:
                    ob = obp.tile([128, 512], F32, tag="ob")
                    if HPC > 1:
                        nc.vector.tensor_add(ob, pu[dct], acc[(ss, dct)])
                    else:
                        nc.scalar.copy(out=ob, in_=pu[dct])
                    nc.sync.dma_start(
                        out=out_rows[ss * 128:(ss + 1) * 128,
                                     dct * 512:(dct + 1) * 512],
                        in_=ob)


# ---------------------------------------------------------------------------
# Host side
# ---------------------------------------------------------------------------

def _rope_perm(hd):
    return np.concatenate([np.arange(0, hd, 2), np.arange(1, hd, 2)])


def prepare_inputs(cfg: Cfg, x, freq_cis, wq_base, wk_base, wv_base, head_a,
                   head_b, q_a, q_b, k_a, k_b, v_a, v_b, wo):
    """Fold LoRA + softmax scale + RoPE permutation into per-core weights."""
    B, S, D, HD, HPC, NC_ = cfg.B, cfg.S, cfg.D, cfg.HD, cfg.HPC, cfg.NCORES
    HEADS = HPC * NC_
    NDB = cfg.NDB
    LORA_SCALE = 2.0
    sm = 1.0 / math.sqrt(HD)

    def fold(w_base, oa, ob):
        w = w_base.astype(np.float64).copy()
        only = LORA_SCALE * (oa.astype(np.float64) @ ob.astype(np.float64))
        hoff = LORA_SCALE * (head_a.astype(np.float64)
                             @ head_b.astype(np.float64))
        w += hoff
        w += np.tile(only, (1, HEADS))
        return w

    wq_eff = fold(wq_base, q_a, q_b) * sm
    wk_eff = fold(wk_base, k_a, k_b)
    wv_eff = fold(wv_base, v_a, v_b)

    perm = _rope_perm(HD)
    for h in range(HEADS):
        cols = h * HD + perm
        wq_eff[:, h * HD:(h + 1) * HD] = wq_eff[:, cols]
        wk_eff[:, h * HD:(h + 1) * HD] = wk_eff[:, cols]
    wq_eff = wq_eff.astype(np.float16)
    wk_eff = wk_eff.astype(np.float16)
    wv_eff = wv_eff.astype(np.float16)

    xT = np.ascontiguousarray(x.reshape(cfg.BS, D).T.astype(np.float16))

    cos = freq_cis[:S, :, 0].T.astype(np.float16)   # [64, S]
    sin = freq_cis[:S, :, 1].T.astype(np.float16)
    cosT = np.ascontiguousarray(np.concatenate([cos, cos], axis=0))
    sinT = np.ascontiguousarray(np.concatenate([sin, sin], axis=0))

    ii = np.arange(128)[:, None]
    cc = np.arange(128)[None, :]
    band = np.where(ii <= cc, 0.0, NEG).astype(np.float32)

    def panelize(w_core):
        # [D, HPC*128] -> [HPC, 128, NDB, 128] (p-major per head panel)
        out = np.empty((HPC, 128, NDB, 128), np.float16)
        for h in range(HPC):
            p = w_core[:, h * 128:(h + 1) * 128]          # [D, 128]
            out[h] = p.reshape(NDB, 128, 128).transpose(1, 0, 2)
        return np.ascontiguousarray(out)

    # wo: h-major block permutation, then p-major panels
    wo16 = wo.astype(np.float16)
    NH = cfg.NHB // HPC
    wo_p = np.empty((HPC, 128, NH, D), np.float16)
    for h in range(HPC):
        for j in range(NH):
            blk = wo16[(j * HPC + h) * 128:(j * HPC + h + 1) * 128, :]
            wo_p[h, :, j, :] = blk
    wo_p = np.ascontiguousarray(wo_p)

    in_maps = []
    for c in range(NC_):
        sl = slice(c * HPC * HD, (c + 1) * HPC * HD)
        in_maps.append(dict(
            xT=xT,
            wq=panelize(wq_eff[:, sl]),
            wk=panelize(wk_eff[:, sl]),
            wv=panelize(wv_eff[:, sl]),
            wo=wo_p,
            cosT=cosT, sinT=sinT, band=band,
        ))
    return in_maps


_BASS_CACHE = {}


def _get_bass(cfg: Cfg):
    key = (cfg.B, cfg.S, cfg.D, cfg.HPC, cfg.NCORES)
    if key not in _BASS_CACHE:
        _BASS_CACHE[key] = build_bass(cfg)
    return _BASS_CACHE[key]


def kernel(**inputs) -> np.ndarray:
    from concourse.bass_utils import run_bass_kernel_spmd

    x = np.asarray(inputs["x"])
    B, S, D = x.shape
    cfg = Cfg(B=B, S=S, D=D, HEADS=16, NCORES=8)
    in_maps = prepare_inputs(cfg, **{k: np.asarray(v)
                                     for k, v in inputs.items()})
    nc = _get_bass(cfg)
    res = run_bass_kernel_spmd(nc, in_maps, list(range(cfg.NCORES)))
    rows = np.concatenate([res.results[c]["out_rows"]
                           for c in range(cfg.NCORES)], axis=0)
    return rows.reshape(B, S, D).astype(np.float32)


# revision 16
# speedup vs baseline: 1.2494x; 1.2494x over previous
"""Trainium2 Bass kernel for LoRA-augmented causal attention.

Reference computation (per nn_Attention_31688268710508):
  x:(B,S,D) -> q/k/v = x@W* + broadcast LoRA + shared head-offset LoRA,
  RoPE(q,k), causal softmax attention per (b,head), out-proj with wo.

Strategy (8 NeuronCores, tensor-parallel over heads):
  * All rank-8 LoRA terms are folded into effective projection weights on
    the host (x @ (A@B) == (x@A)@B up to fp rounding).  The softmax
    1/sqrt(HD) scale is folded into Wq.
  * RoPE pairs (2i,2i+1) are moved to (i, i+64) by permuting Wq/Wk columns
    (scores are invariant to a shared q/k head-dim permutation), making
    RoPE a half-partition-block rotation in the on-chip [hd, s] layout.
  * fp16 operands everywhere on the PE (fp32 PSUM accumulation).
  * Projections (W stationary, x^T streamed) and causal attention
    (S^T = K^T.T Q^T blocks -> exp -> fp16 P^T -> V-stationary O^T
    matmuls; softmax denominator via DVE-accumulated P^T + one ones-matmul)
    are emitted INTERLEAVED so projection matmuls for (b,h) segment k+1
    hide the exp latency of attention segment k and keep the PE dense/warm.
  * Per-head AllToAll redistributes O^T by sequence rows; the first
    exchange overlaps the second head's attention.  Each core then
    multiplies with full wo for its 512-row output shard.
  * DMA channels: qSP (sync) streams x^T slabs in per-db chunks; qAct
    (scalar) carries weight panels / tables / wo / a2a-out / out_rows;
    gpsimd SWDGE carries compute-dependent a2a-in stores so they never
    head-of-line-block a load queue.  Weight panels and wo are pre-laid
    out on the host as [128, nblk, 128] so their DMAs are contiguous.
  * wo's h=0 half is preloaded into an outer SBUF tile during attention
    segment 3 so the out-projection starts the moment AllToAll(h=0)
    lands; the h=1 half streams in under the h=0 matmuls.
"""

import math
import os
import sys
from contextlib import ExitStack

import numpy as np

for _p in ("/opt/trn_rl_repo", "/root/.axon_site/_ro/trn_rl_repo"):
    if os.path.isdir(_p) and _p not in sys.path:
        sys.path.insert(0, _p)

import concourse.bass as bass  # noqa: E402
import concourse.mybir as mybir  # noqa: E402
import concourse.tile as tile  # noqa: E402
from concourse import bacc  # noqa: E402
from concourse.masks import make_identity  # noqa: E402

F32 = mybir.dt.float32
F16 = mybir.dt.float16
EXP = mybir.ActivationFunctionType.Exp

NEG = -1.0e30


class Cfg:
    def __init__(self, B=2, S=2048, D=2048, HEADS=16, NCORES=8):
        self.B, self.S, self.D, self.NCORES = B, S, D, NCORES
        self.HD = 128
        self.HPC = HEADS // NCORES          # heads per core
        self.BS = B * S
        self.RPC = self.BS // NCORES        # output rows per core
        self.ST = 512                       # free-dim tile (q tile, s tile)
        self.SG = min(512, self.S)          # x^T slab width (s cols)
        self.NDB = D // 128                 # contraction blocks
        self.GPB = self.S // self.SG        # slabs per batch b
        self.NST_G = self.SG // self.ST     # s tiles per slab
        self.KPG = 2                        # k-blocks per exp group
        self.NHB = (HEADS * self.HD) // 128  # out-proj contraction blocks
        assert self.HD == 128 and D % 512 == 0 and S % self.ST == 0
        assert self.RPC % 128 == 0 and self.SG % self.ST == 0


def build_bass(cfg: Cfg) -> bass.Bass:
    B, S, D = cfg.B, cfg.S, cfg.D
    HPC, HD, ST, SG = cfg.HPC, cfg.HD, cfg.ST, cfg.SG
    NC_, RPC = cfg.NCORES, cfg.RPC

    nc = bacc.Bacc("TRN2", target_bir_lowering=False, debug=False,
                   num_devices=NC_)

    xT = nc.declare_dram_parameter("xT", [D, cfg.BS], F16, isOutput=False)
    # weight panels pre-laid out host-side: [HPC, 128, NDB, 128] per proj
    wq = nc.declare_dram_parameter("wq", [HPC, 128, cfg.NDB, 128], F16,
                                   isOutput=False)
    wk = nc.declare_dram_parameter("wk", [HPC, 128, cfg.NDB, 128], F16,
                                   isOutput=False)
    wv = nc.declare_dram_parameter("wv", [HPC, 128, cfg.NDB, 128], F16,
                                   isOutput=False)
    # wo pre-permuted h-major: [HPC, 128, NHB//HPC, D]
    wo = nc.declare_dram_parameter("wo", [HPC, 128, cfg.NHB // HPC, D], F16,
                                   isOutput=False)
    cosT = nc.declare_dram_parameter("cosT", [128, S], F16, isOutput=False)
    sinT = nc.declare_dram_parameter("sinT", [128, S], F16, isOutput=False)
    band = nc.declare_dram_parameter("band", [128, 128], F32,
                                     isOutput=False)
    out_rows = nc.declare_dram_parameter("out_rows", [RPC, D], F32,
                                         isOutput=True)

    a2a_ins = [nc.dram_tensor(f"a2a_in{h}", [NC_, HD, RPC], F16)
               for h in range(HPC)]
    a2a_outs = [nc.dram_tensor(f"a2a_out{h}", [NC_, HD, RPC], F16)
                for h in range(HPC)]

    with tile.TileContext(nc) as tc:
        with ExitStack() as ctx:
            constp = ctx.enter_context(tc.tile_pool(name="const", bufs=1))
            ident = constp.tile([128, 128], F16)
            make_identity(nc, ident)
            band_sb = constp.tile([128, 128], F32)
            ones_sb = constp.tile([128, 128], F16)
            nc.vector.memset(ones_sb, 1.0)

            qtp = ctx.enter_context(tc.tile_pool(name="qt", bufs=1))
            ktp = ctx.enter_context(tc.tile_pool(name="kt", bufs=1))
            vhp = ctx.enter_context(tc.tile_pool(name="vhat", bufs=1))
            wosp = ctx.enter_context(tc.tile_pool(name="wo_sb", bufs=1))
            QT = qtp.tile([128, HPC, cfg.BS], F16)
            KT = ktp.tile([128, HPC, cfg.BS], F16)
            Vhat = vhp.tile([128, HPC, B, S // 128, 128], F16)
            WO = wosp.tile([128, cfg.NHB // HPC, D], F16)

            st = _State(nc, tc, cfg, xT, (wq, wk, wv), wo, WO, cosT, sinT,
                        band, band_sb, ones_sb, ident, QT, KT, Vhat,
                        a2a_ins, a2a_outs)
            with ExitStack() as phase_ctx:
                st.open_phase_pools(phase_ctx)

                segs = [(b, h) for b in range(B) for h in range(HPC)]
                # lag-1 interleave: attention segment k runs against
                # projection segment k+1's matmuls.
                pgens = [st.proj_segment(b, h) for (b, h) in segs]
                agens = [st.attn_segment(b, h) for (b, h) in segs]
                for _ in pgens[0]:
                    pass
                for k, ag in enumerate(agens):
                    # preload wo's h=0 half while attention still runs,
                    # one 512KB sub-chunk per yield so weight panels are
                    # not delayed in the qAct FIFO
                    wo_feed = (iter(range(cfg.NHB // HPC))
                               if k == len(segs) - 2 else iter(()))
                    pg = pgens[k + 1] if k + 1 < len(segs) else None
                    for _ in ag:
                        if pg is not None:
                            next(pg, None)
                        j = next(wo_feed, None)
                        if j is not None:
                            nc.sync.dma_start(out=st.WO[:, j, :],
                                              in_=st.wo_dram[0, :, j, :])
                    if pg is not None:
                        for _ in pg:
                            pass
                    b, h = segs[k]
                    if b == B - 1:
                        nc.gpsimd.collective_compute(
                            "AllToAll",
                            mybir.AluOpType.bypass,
                            replica_groups=[list(range(NC_))],
                            ins=[a2a_ins[h][:, :, :]],
                            outs=[a2a_outs[h][:, :, :]],
                        )

            with (
                tc.tile_pool(name="aot_sb", bufs=1) as aip,
                tc.tile_pool(name="accp", bufs=1) as accp,
                tc.tile_pool(name="ob", bufs=4) as obp,
                tc.tile_pool(name="psU", bufs=8, space="PSUM") as psU,
            ):
                _outproj(nc, cfg, st, a2a_outs, aip, accp, obp, psU,
                         out_rows)

    nc.finalize()
    return nc


class _State:
    """Shared emission state for the interleaved proj/attention phases."""

    def __init__(self, nc, tc, cfg, xT, w_drams, wo_dram, WO, cosT, sinT,
                 band, band_sb, ones_sb, ident, QT, KT, Vhat, a2a_ins,
                 a2a_outs):
        self.nc, self.tc, self.cfg = nc, tc, cfg
        self.xT, self.w_drams = xT, w_drams
        self.wo_dram, self.WO = wo_dram, WO
        self.cosT, self.sinT, self.band = cosT, sinT, band
        self.band_sb, self.ones_sb, self.ident = band_sb, ones_sb, ident
        self.QT, self.KT, self.Vhat = QT, KT, Vhat
        self.a2a_ins, self.a2a_outs = a2a_ins, a2a_outs
        self.slabs = {}          # g -> slab tile
        self.tables_loaded = False

    def open_phase_pools(self, ctx):
        tc = self.tc
        self.xp = ctx.enter_context(tc.tile_pool(name="xslab", bufs=4))
        self.wp = ctx.enter_context(tc.tile_pool(name="wres", bufs=1))
        self.wpanels = {}
        self.tbp = ctx.enter_context(tc.tile_pool(name="tables", bufs=1))
        self.rp = ctx.enter_context(tc.tile_pool(name="ropet", bufs=3))
        self.stp = ctx.enter_context(tc.tile_pool(name="vstage", bufs=3))
        self.ptp = ctx.enter_context(tc.tile_pool(name="ptile", bufs=3))
        self.pap = ctx.enter_context(tc.tile_pool(name="pacc", bufs=2))
        self.aotp = ctx.enter_context(tc.tile_pool(name="aot", bufs=3))
        self.rcp = ctx.enter_context(tc.tile_pool(name="rec", bufs=2))
        self.psS = ctx.enter_context(
            tc.tile_pool(name="psS", bufs=3, space="PSUM"))
        self.psD = ctx.enter_context(
            tc.tile_pool(name="psD", bufs=1, space="PSUM"))
        self.psJ = ctx.enter_context(
            tc.tile_pool(name="psJ", bufs=2, space="PSUM"))
        self.psO = ctx.enter_context(
            tc.tile_pool(name="psO", bufs=1, space="PSUM"))
        self.psX = ctx.enter_context(
            tc.tile_pool(name="psX", bufs=1, space="PSUM"))
        self.cos_sb = self.tbp.tile([128, self.cfg.S], F16)
        self.sin_sb = self.tbp.tile([128, self.cfg.S], F16)

    def load_wo_chunk(self, h):
        """Stream wo's half for head h into the resident WO tile (qAct)."""
        nc, cfg = self.nc, self.cfg
        nh = cfg.NHB // cfg.HPC
        for j in range(nh):
            nc.sync.dma_start(out=self.WO[:, j, :],
                              in_=self.wo_dram[h, :, j, :])

    def _wpanel(self, proj, h):
        """Resident weight panel [128, NDB, 128], loaded once (qAct)."""
        key = (proj, h)
        if key not in self.wpanels:
            t = self.wp.tile([128, self.cfg.NDB, 128], F16,
                             tag=f"w{proj}_{h}", name=f"w{proj}_{h}")
            nc = self.nc
            for c in range(0, self.cfg.NDB, 4):
                nc.scalar.dma_start(out=t[:, c:c + 4, :],
                                    in_=self.w_drams[proj][h, :, c:c + 4, :])
            self.wpanels[key] = t
        return self.wpanels[key]

    def _slab(self, g):
        """Whole-slab x^T tile [128, NDB, SG]; chunked DMAs (qSP)."""
        nc, cfg = self.nc, self.cfg
        if g not in self.slabs:
            t = self.xp.tile([128, cfg.NDB, cfg.SG], F16, tag="xs",
                             name=f"xs{g}")
            src = self.xT[:, g * cfg.SG:(g + 1) * cfg.SG].rearrange(
                "(db p) c -> p db c", p=128)
            if g == 0:
                # fine chunks so the first matmuls start after ~128KB
                for db in range(4):
                    nc.sync.dma_start(out=t[:, db, :], in_=src[:, db, :])
                for c in range(1, 4):
                    nc.sync.dma_start(out=t[:, 4 * c:4 * (c + 1), :],
                                      in_=src[:, 4 * c:4 * (c + 1), :])
            else:
                for c in range(4):
                    nc.sync.dma_start(out=t[:, 4 * c:4 * (c + 1), :],
                                      in_=src[:, 4 * c:4 * (c + 1), :])
            self.slabs[g] = t
            if not self.tables_loaded:
                self.tables_loaded = True
                nc.sync.dma_start(out=self.band_sb, in_=self.band[:, :])
                nc.sync.dma_start(out=self.cos_sb, in_=self.cosT[:, :])
                nc.sync.dma_start(out=self.sin_sb, in_=self.sinT[:, :])
        return self.slabs[g]

    def proj_segment(self, b, h):
        """Generator: projections (Q,K,V) of head h over batch b's slabs.

        Yields after each contraction step (NST_G matmuls) so the driver
        can interleave attention work.
        """
        nc, cfg = self.nc, self.cfg
        S, SG, ST, NDB = cfg.S, cfg.SG, cfg.ST, cfg.NDB
        for g in range(b * cfg.GPB, (b + 1) * cfg.GPB):
            xs = self._slab(g)
            for proj in range(3):
                w_t = self._wpanel(proj, h)
                accs = [self.psJ.tile([128, ST], F32, tag="pj",
                                      name=f"pj{st_i}")
                        for st_i in range(cfg.NST_G)]
                for db in range(NDB):
                    for st_i in range(cfg.NST_G):
                        nc.tensor.matmul(
                            accs[st_i],
                            lhsT=w_t[:, db, :],
                            rhs=xs[:, db, st_i * ST:(st_i + 1) * ST],
                            start=(db == 0), stop=(db == NDB - 1))
                    yield
                for st_i in range(cfg.NST_G):
                    gcol = g * SG + st_i * ST
                    scol = gcol % S
                    acc = accs[st_i]
                    if proj < 2:
                        dst = (self.QT if proj == 0 else
                               self.KT)[:, h, gcol:gcol + ST]
                        t1 = self.rp.tile([128, ST], F32, tag="t1")
                        nc.vector.tensor_mul(
                            t1, acc, self.cos_sb[:, scol:scol + ST])
                        t2 = self.rp.tile([128, ST], F32, tag="t2")
                        nc.vector.tensor_mul(
                            t2[0:64], acc[64:128],
                            self.sin_sb[0:64, scol:scol + ST])
                        nc.vector.tensor_mul(
                            t2[64:128], acc[0:64],
                            self.sin_sb[64:128, scol:scol + ST])
                        nc.vector.tensor_sub(dst[0:64], t1[0:64], t2[0:64])
                        nc.vector.tensor_add(dst[64:128], t1[64:128],
                                             t2[64:128])
                    else:
                        vstage = self.stp.tile([128, ST], F16, tag="vst")
                        nc.scalar.copy(out=vstage, in_=acc)
                        for jj in range(ST // 128):
                            gc = gcol + jj * 128
                            b_idx, kblk = gc // S, (gc % S) // 128
                            pst = self.psX.tile([128, 128], F16, tag="px",
                                                name="pst")
                            nc.tensor.transpose(
                                pst, vstage[:, jj * 128:(jj + 1) * 128],
                                self.ident)
                            nc.scalar.copy(
                                out=self.Vhat[:, h, b_idx, kblk, :],
                                in_=pst)
                    yield

    def attn_segment(self, b, h):
        """Generator: causal attention for (h, b).

        KPG-free pipeline: scores for k-block i+2 are emitted before the
        PV of block i (lookahead 2) so the exp latency hides under queued
        matmuls.  Diagonal blocks compute only columns [d, ST) with a
        fixed 128-wide triangle mask; the excluded pt region is zeroed
        off the critical chain.
        """
        nc, cfg = self.nc, self.cfg
        S, ST, RPC = cfg.S, cfg.ST, cfg.RPC
        LA = 2
        tri = self.band_sb
        for qt in range(S // ST):
            q0 = qt * ST
            nkb = (q0 + ST) // 128
            po = self.psO.tile([128, ST], F32, tag="po")
            pa = self.pap.tile([128, ST], F16, tag="pa")
            state = {}
            pa_state = {}

            def emit_sc(kb):
                d = kb * 128 - q0
                lo = max(d, 0)
                ps = self.psS.tile([128, ST], F32, tag="ps")
                pt = self.ptp.tile([128, ST], F16, tag="pt")
                nc.tensor.matmul(
                    ps[:, lo:ST],
                    lhsT=self.KT[:, h, b * S + kb * 128:
                                 b * S + kb * 128 + 128],
                    rhs=self.QT[:, h, b * S + q0 + lo:b * S + q0 + ST],
                    start=True, stop=True)
                if lo > 0:
                    nc.vector.memset(pt[:, 0:lo], 0.0)
                if d >= 0:
                    nc.vector.tensor_add(ps[:, lo:lo + 128],
                                         ps[:, lo:lo + 128], tri)
                nc.scalar.activation(pt[:, lo:ST], ps[:, lo:ST], EXP)
                if kb > 0:
                    flush_pa(kb - 1)
                state[kb] = (pt, lo)
                pa_state[kb] = (pt, lo)

            def flush_pa(kb):
                pt, lo = pa_state.pop(kb)
                if kb == 0:
                    nc.vector.tensor_copy(pa, pt)
                else:
                    nc.vector.tensor_add(pa[:, lo:ST], pa[:, lo:ST],
                                         pt[:, lo:ST])

            def emit_pv(kb):
                pt, lo = state.pop(kb)
                nc.tensor.matmul(
                    po[:, lo:ST] if lo > 0 else po,
                    lhsT=self.Vhat[:, h, b, kb, :],
                    rhs=pt[:, lo:ST],
                    start=(kb == 0), stop=(kb == nkb - 1))

            for i in range(nkb + LA):
                if i < nkb:
                    emit_sc(i)
                if i >= LA:
                    emit_pv(i - LA)
                if i % 2 == 1 or i >= nkb:
                    yield
            flush_pa(nkb - 1)
            pd = self.psD.tile([128, ST], F32, tag="pd")
            nc.tensor.matmul(pd, lhsT=self.ones_sb, rhs=pa,
                             start=True, stop=True)
            rec = self.rcp.tile([128, ST], F32, tag="rec")
            nc.vector.reciprocal_approx_fast(out=rec, in_=pd)
            aot = self.aotp.tile([128, ST], F16, tag="aot")
            nc.vector.tensor_mul(aot, po, rec)
            CSZ = min(ST, RPC)
            for ci in range(ST // CSZ):
                r0 = b * S + q0 + ci * CSZ
                nc.scalar.dma_start(
                    out=self.a2a_ins[h][r0 // RPC, :,
                                        r0 % RPC:r0 % RPC + CSZ],
                    in_=aot[:, ci * CSZ:(ci + 1) * CSZ])


def _outproj(nc, cfg, st, a2a_outs, aip, accp, obp, psU, out_rows):
    """out_rows[s', :] = AO^T_full[:, s'].T @ wo, contraction over heads.

    One PSUM round per (h, ss): round h accumulates that head's 8 blocks.
    The h=0 half of wo is already resident (preloaded during attention);
    the h=1 half streams in under the h=0 matmuls.  h=0 partial sums park
    in SBUF and are added during the h=1 pass.
    """
    D, HPC, RPC = cfg.D, cfg.HPC, cfg.RPC
    NH = cfg.NHB // HPC                   # blocks per head-group (8)
    NDC = D // 512
    NSS = RPC // 128

    # a2a_out tiles; h>0 loads emitted late (with that h's wo reload) so
    # the qSP FIFO order is [aip h0][wo h1][aip h1][out_rows]
    aot_sb = {}

    def load_aip(h):
        for j in range(NH):
            a = aip.tile([128, RPC], F16, tag=f"ai{h}_{j}",
                         name=f"ai_sb{h}_{j}")
            nc.sync.dma_start(out=a, in_=a2a_outs[h][j, :, :])
            aot_sb[(h, j)] = a

    load_aip(0)
    def wo_rhs(h, j, c0, c1):
        if h == 0:
            return st.WO[:, j, c0:c1]
        # h=1 wo blocks live in the dead QT/KT space: panels 0-3 in QT,
        # 4-7 in KT, flat-addressed [hh, 2048-col half]
        tile_ = st.QT if j < 4 else st.KT
        base = ((j % 4) % 2) * 2048
        return tile_[:, (j % 4) // 2, base + c0:base + c1]

    acc = {}
    for h in range(HPC):
        if h > 0:
            # emitted AFTER attention (QT/KT last reads) so the WAR frees
            # immediately; these loads hide under h=0's matmuls
            for j in range(NH):
                nc.sync.dma_start(out=wo_rhs(h, j, 0, D),
                                  in_=st.wo_dram[h, :, j, :])
            load_aip(h)
        for ss in range(NSS):
            pu = [psU.tile([128, 512], F32, tag="pu", name=f"pu_{dct}")
                  for dct in range(NDC)]
            for j in range(NH):
                for dct in range(NDC):
                    nc.tensor.matmul(
                        pu[dct],
                        lhsT=aot_sb[(h, j)][:, ss * 128:(ss + 1) * 128],
                        rhs=wo_rhs(h, j, dct * 512, (dct + 1) * 512),
                        start=(j == 0), stop=(j == NH - 1))
            for dct in range(NDC):
                if h == 0 and HPC > 1:
                    t = accp.tile([128, 512], F32, tag=f"acc{ss}_{dct}",
                                  name=f"acc{ss}_{dct}")
                    nc.vector.tensor_copy(t, pu[dct])
                    acc[(ss, dct)] = t
                else:
                    ob = obp.tile([128, 512], F32, tag="ob")
                    if HPC > 1:
                        nc.vector.tensor_add(ob, pu[dct], acc[(ss, dct)])
                    else:
                        nc.scalar.copy(out=ob, in_=pu[dct])
                    nc.sync.dma_start(
                        out=out_rows[ss * 128:(ss + 1) * 128,
                                     dct * 512:(dct + 1) * 512],
                        in_=ob)


# ---------------------------------------------------------------------------
# Host side
# ---------------------------------------------------------------------------

def _rope_perm(hd):
    return np.concatenate([np.arange(0, hd, 2), np.arange(1, hd, 2)])


def prepare_inputs(cfg: Cfg, x, freq_cis, wq_base, wk_base, wv_base, head_a,
                   head_b, q_a, q_b, k_a, k_b, v_a, v_b, wo):
    """Fold LoRA + softmax scale + RoPE permutation into per-core weights."""
    B, S, D, HD, HPC, NC_ = cfg.B, cfg.S, cfg.D, cfg.HD, cfg.HPC, cfg.NCORES
    HEADS = HPC * NC_
    NDB = cfg.NDB
    LORA_SCALE = 2.0
    sm = 1.0 / math.sqrt(HD)

    def fold(w_base, oa, ob):
        w = w_base.astype(np.float64).copy()
        only = LORA_SCALE * (oa.astype(np.float64) @ ob.astype(np.float64))
        hoff = LORA_SCALE * (head_a.astype(np.float64)
                             @ head_b.astype(np.float64))
        w += hoff
        w += np.tile(only, (1, HEADS))
        return w

    wq_eff = fold(wq_base, q_a, q_b) * sm
    wk_eff = fold(wk_base, k_a, k_b)
    wv_eff = fold(wv_base, v_a, v_b)

    perm = _rope_perm(HD)
    for h in range(HEADS):
        cols = h * HD + perm
        wq_eff[:, h * HD:(h + 1) * HD] = wq_eff[:, cols]
        wk_eff[:, h * HD:(h + 1) * HD] = wk_eff[:, cols]
    wq_eff = wq_eff.astype(np.float16)
    wk_eff = wk_eff.astype(np.float16)
    wv_eff = wv_eff.astype(np.float16)

    xT = np.ascontiguousarray(x.reshape(cfg.BS, D).T.astype(np.float16))

    cos = freq_cis[:S, :, 0].T.astype(np.float16)   # [64, S]
    sin = freq_cis[:S, :, 1].T.astype(np.float16)
    cosT = np.ascontiguousarray(np.concatenate([cos, cos], axis=0))
    sinT = np.ascontiguousarray(np.concatenate([sin, sin], axis=0))

    ii = np.arange(128)[:, None]
    cc = np.arange(128)[None, :]
    band = np.where(ii <= cc, 0.0, NEG).astype(np.float32)

    def panelize(w_core):
        # [D, HPC*128] -> [HPC, 128, NDB, 128] (p-major per head panel)
        out = np.empty((HPC, 128, NDB, 128), np.float16)
        for h in range(HPC):
            p = w_core[:, h * 128:(h + 1) * 128]          # [D, 128]
            out[h] = p.reshape(NDB, 128, 128).transpose(1, 0, 2)
        return np.ascontiguousarray(out)

    # wo: h-major block permutation, then p-major panels
    wo16 = wo.astype(np.float16)
    NH = cfg.NHB // HPC
    wo_p = np.empty((HPC, 128, NH, D), np.float16)
    for h in range(HPC):
        for j in range(NH):
            blk = wo16[(j * HPC + h) * 128:(j * HPC + h + 1) * 128, :]
            wo_p[h, :, j, :] = blk
    wo_p = np.ascontiguousarray(wo_p)

    in_maps = []
    for c in range(NC_):
        sl = slice(c * HPC * HD, (c + 1) * HPC * HD)
        in_maps.append(dict(
            xT=xT,
            wq=panelize(wq_eff[:, sl]),
            wk=panelize(wk_eff[:, sl]),
            wv=panelize(wv_eff[:, sl]),
            wo=wo_p,
            cosT=cosT, sinT=sinT, band=band,
        ))
    return in_maps


_BASS_CACHE = {}


def _get_bass(cfg: Cfg):
    key = (cfg.B, cfg.S, cfg.D, cfg.HPC, cfg.NCORES)
    if key not in _BASS_CACHE:
        _BASS_CACHE[key] = build_bass(cfg)
    return _BASS_CACHE[key]


def kernel(**inputs) -> np.ndarray:
    from concourse.bass_utils import run_bass_kernel_spmd

    x = np.asarray(inputs["x"])
    B, S, D = x.shape
    cfg = Cfg(B=B, S=S, D=D, HEADS=16, NCORES=8)
    in_maps = prepare_inputs(cfg, **{k: np.asarray(v)
                                     for k, v in inputs.items()})
    nc = _get_bass(cfg)
    res = run_bass_kernel_spmd(nc, in_maps, list(range(cfg.NCORES)))
    rows = np.concatenate([res.results[c]["out_rows"]
                           for c in range(cfg.NCORES)], axis=0)
    return rows.reshape(B, S, D).astype(np.float32)


# revision 17
# speedup vs baseline: 1.2598x; 1.0084x over previous
"""Trainium2 Bass kernel for LoRA-augmented causal attention.

Reference computation (per nn_Attention_31688268710508):
  x:(B,S,D) -> q/k/v = x@W* + broadcast LoRA + shared head-offset LoRA,
  RoPE(q,k), causal softmax attention per (b,head), out-proj with wo.

Strategy (8 NeuronCores, tensor-parallel over heads):
  * All rank-8 LoRA terms are folded into effective projection weights on
    the host (x @ (A@B) == (x@A)@B up to fp rounding).  The softmax
    1/sqrt(HD) scale is folded into Wq.
  * RoPE pairs (2i,2i+1) are moved to (i, i+64) by permuting Wq/Wk columns
    (scores are invariant to a shared q/k head-dim permutation), making
    RoPE a half-partition-block rotation in the on-chip [hd, s] layout.
  * fp16 operands everywhere on the PE (fp32 PSUM accumulation).
  * Projections (W stationary, x^T streamed) and causal attention
    (S^T = K^T.T Q^T blocks -> exp -> fp16 P^T -> V-stationary O^T
    matmuls; softmax denominator via DVE-accumulated P^T + one ones-matmul)
    are emitted INTERLEAVED so projection matmuls for (b,h) segment k+1
    hide the exp latency of attention segment k and keep the PE dense/warm.
  * Per-head AllToAll redistributes O^T by sequence rows; the first
    exchange overlaps the second head's attention.  Each core then
    multiplies with full wo for its 512-row output shard.
  * DMA channels: qSP (sync) streams x^T slabs in per-db chunks; qAct
    (scalar) carries weight panels / tables / wo / a2a-out / out_rows;
    gpsimd SWDGE carries compute-dependent a2a-in stores so they never
    head-of-line-block a load queue.  Weight panels and wo are pre-laid
    out on the host as [128, nblk, 128] so their DMAs are contiguous.
  * wo's h=0 half is preloaded into an outer SBUF tile during attention
    segment 3 so the out-projection starts the moment AllToAll(h=0)
    lands; the h=1 half streams in under the h=0 matmuls.
"""

import math
import os
import sys
from contextlib import ExitStack

import numpy as np

for _p in ("/opt/trn_rl_repo", "/root/.axon_site/_ro/trn_rl_repo"):
    if os.path.isdir(_p) and _p not in sys.path:
        sys.path.insert(0, _p)

import concourse.bass as bass  # noqa: E402
import concourse.mybir as mybir  # noqa: E402
import concourse.tile as tile  # noqa: E402
from concourse import bacc  # noqa: E402
from concourse.masks import make_identity  # noqa: E402

F32 = mybir.dt.float32
F16 = mybir.dt.float16
EXP = mybir.ActivationFunctionType.Exp

NEG = -1.0e30


class Cfg:
    def __init__(self, B=2, S=2048, D=2048, HEADS=16, NCORES=8):
        self.B, self.S, self.D, self.NCORES = B, S, D, NCORES
        self.HD = 128
        self.HPC = HEADS // NCORES          # heads per core
        self.BS = B * S
        self.RPC = self.BS // NCORES        # output rows per core
        self.ST = 512                       # free-dim tile (q tile, s tile)
        self.SG = min(512, self.S)          # x^T slab width (s cols)
        self.NDB = D // 128                 # contraction blocks
        self.GPB = self.S // self.SG        # slabs per batch b
        self.NST_G = self.SG // self.ST     # s tiles per slab
        self.KPG = 2                        # k-blocks per exp group
        self.NHB = (HEADS * self.HD) // 128  # out-proj contraction blocks
        assert self.HD == 128 and D % 512 == 0 and S % self.ST == 0
        assert self.RPC % 128 == 0 and self.SG % self.ST == 0


def build_bass(cfg: Cfg) -> bass.Bass:
    B, S, D = cfg.B, cfg.S, cfg.D
    HPC, HD, ST, SG = cfg.HPC, cfg.HD, cfg.ST, cfg.SG
    NC_, RPC = cfg.NCORES, cfg.RPC

    nc = bacc.Bacc("TRN2", target_bir_lowering=False, debug=False,
                   num_devices=NC_)

    xT = nc.declare_dram_parameter("xT", [D, cfg.BS], F16, isOutput=False)
    # weight panels pre-laid out host-side: [HPC, 128, NDB, 128] per proj
    wq = nc.declare_dram_parameter("wq", [HPC, 128, cfg.NDB, 128], F16,
                                   isOutput=False)
    wk = nc.declare_dram_parameter("wk", [HPC, 128, cfg.NDB, 128], F16,
                                   isOutput=False)
    wv = nc.declare_dram_parameter("wv", [HPC, 128, cfg.NDB, 128], F16,
                                   isOutput=False)
    # wo pre-permuted h-major: [HPC, 128, NHB//HPC, D]
    wo = nc.declare_dram_parameter("wo", [HPC, 128, cfg.NHB // HPC, D], F16,
                                   isOutput=False)
    cosT = nc.declare_dram_parameter("cosT", [128, S], F16, isOutput=False)
    sinT = nc.declare_dram_parameter("sinT", [128, S], F16, isOutput=False)
    band = nc.declare_dram_parameter("band", [128, 128], F32,
                                     isOutput=False)
    out_rows = nc.declare_dram_parameter("out_rows", [RPC, D], F32,
                                         isOutput=True)

    a2a_ins = [nc.dram_tensor(f"a2a_in{h}", [NC_, HD, RPC], F16)
               for h in range(HPC)]
    a2a_outs = [nc.dram_tensor(f"a2a_out{h}", [NC_, HD, RPC], F16)
                for h in range(HPC)]

    with tile.TileContext(nc) as tc:
        with ExitStack() as ctx:
            constp = ctx.enter_context(tc.tile_pool(name="const", bufs=1))
            ident = constp.tile([128, 128], F16)
            make_identity(nc, ident)
            band_sb = constp.tile([128, 128], F32)
            ones_sb = constp.tile([128, 128], F16)
            nc.vector.memset(ones_sb, 1.0)

            qtp = ctx.enter_context(tc.tile_pool(name="qt", bufs=1))
            ktp = ctx.enter_context(tc.tile_pool(name="kt", bufs=1))
            vhp = ctx.enter_context(tc.tile_pool(name="vhat", bufs=1))
            wosp = ctx.enter_context(tc.tile_pool(name="wo_sb", bufs=1))
            QT = qtp.tile([128, HPC, cfg.BS], F16)
            KT = ktp.tile([128, HPC, cfg.BS], F16)
            Vhat = vhp.tile([128, HPC, B, S // 128, 128], F16)
            WO = wosp.tile([128, cfg.NHB // HPC, D], F16)

            st = _State(nc, tc, cfg, xT, (wq, wk, wv), wo, WO, cosT, sinT,
                        band, band_sb, ones_sb, ident, QT, KT, Vhat,
                        a2a_ins, a2a_outs)
            with ExitStack() as phase_ctx:
                st.open_phase_pools(phase_ctx)

                segs = [(b, h) for b in range(B) for h in range(HPC)]
                # lag-1 interleave: attention segment k runs against
                # projection segment k+1's matmuls.
                pgens = [st.proj_segment(b, h) for (b, h) in segs]
                agens = [st.attn_segment(b, h) for (b, h) in segs]
                for _ in pgens[0]:
                    pass
                for k, ag in enumerate(agens):
                    # preload wo's h=0 half while attention still runs,
                    # one 512KB sub-chunk per yield so weight panels are
                    # not delayed in the qAct FIFO
                    wo_feed = (iter(range(cfg.NHB // HPC))
                               if k == len(segs) - 2 else iter(()))
                    pg = pgens[k + 1] if k + 1 < len(segs) else None
                    for _ in ag:
                        if pg is not None:
                            next(pg, None)
                        j = next(wo_feed, None)
                        if j is not None:
                            nc.sync.dma_start(out=st.WO[:, j, :],
                                              in_=st.wo_dram[0, :, j, :])
                    if pg is not None:
                        for _ in pg:
                            pass
                    b, h = segs[k]
                    if b == B - 1:
                        nc.gpsimd.collective_compute(
                            "AllToAll",
                            mybir.AluOpType.bypass,
                            replica_groups=[list(range(NC_))],
                            ins=[a2a_ins[h][:, :, :]],
                            outs=[a2a_outs[h][:, :, :]],
                        )

            with (
                tc.tile_pool(name="aot_sb", bufs=1) as aip,
                tc.tile_pool(name="accp", bufs=1) as accp,
                tc.tile_pool(name="ob", bufs=4) as obp,
                tc.tile_pool(name="psU", bufs=8, space="PSUM") as psU,
            ):
                _outproj(nc, cfg, st, a2a_outs, aip, accp, obp, psU,
                         out_rows)

    nc.finalize()
    return nc


class _State:
    """Shared emission state for the interleaved proj/attention phases."""

    def __init__(self, nc, tc, cfg, xT, w_drams, wo_dram, WO, cosT, sinT,
                 band, band_sb, ones_sb, ident, QT, KT, Vhat, a2a_ins,
                 a2a_outs):
        self.nc, self.tc, self.cfg = nc, tc, cfg
        self.xT, self.w_drams = xT, w_drams
        self.wo_dram, self.WO = wo_dram, WO
        self.cosT, self.sinT, self.band = cosT, sinT, band
        self.band_sb, self.ones_sb, self.ident = band_sb, ones_sb, ident
        self.QT, self.KT, self.Vhat = QT, KT, Vhat
        self.a2a_ins, self.a2a_outs = a2a_ins, a2a_outs
        self.slabs = {}          # g -> slab tile
        self.tables_loaded = False

    def open_phase_pools(self, ctx):
        tc = self.tc
        self.xp = ctx.enter_context(tc.tile_pool(name="xslab", bufs=4))
        self.wp = ctx.enter_context(tc.tile_pool(name="wres", bufs=1))
        self.wpanels = {}
        self.tbp = ctx.enter_context(tc.tile_pool(name="tables", bufs=1))
        self.rp = ctx.enter_context(tc.tile_pool(name="ropet", bufs=3))
        self.stp = ctx.enter_context(tc.tile_pool(name="vstage", bufs=3))
        self.ptp = ctx.enter_context(tc.tile_pool(name="ptile", bufs=3))
        self.pap = ctx.enter_context(tc.tile_pool(name="pacc", bufs=2))
        self.aotp = ctx.enter_context(tc.tile_pool(name="aot", bufs=3))
        self.rcp = ctx.enter_context(tc.tile_pool(name="rec", bufs=2))
        self.psS = ctx.enter_context(
            tc.tile_pool(name="psS", bufs=3, space="PSUM"))
        self.psD = ctx.enter_context(
            tc.tile_pool(name="psD", bufs=1, space="PSUM"))
        self.psJ = ctx.enter_context(
            tc.tile_pool(name="psJ", bufs=2, space="PSUM"))
        self.psO = ctx.enter_context(
            tc.tile_pool(name="psO", bufs=1, space="PSUM"))
        self.psX = ctx.enter_context(
            tc.tile_pool(name="psX", bufs=1, space="PSUM"))
        self.cos_sb = self.tbp.tile([128, self.cfg.S], F16)
        self.sin_sb = self.tbp.tile([128, self.cfg.S], F16)

    def load_wo_chunk(self, h):
        """Stream wo's half for head h into the resident WO tile (qAct)."""
        nc, cfg = self.nc, self.cfg
        nh = cfg.NHB // cfg.HPC
        for j in range(nh):
            nc.sync.dma_start(out=self.WO[:, j, :],
                              in_=self.wo_dram[h, :, j, :])

    def _wpanel(self, proj, h):
        """Resident weight panel [128, NDB, 128], loaded once (qAct)."""
        key = (proj, h)
        if key not in self.wpanels:
            t = self.wp.tile([128, self.cfg.NDB, 128], F16,
                             tag=f"w{proj}_{h}", name=f"w{proj}_{h}")
            nc = self.nc
            for c in range(0, self.cfg.NDB, 4):
                nc.scalar.dma_start(out=t[:, c:c + 4, :],
                                    in_=self.w_drams[proj][h, :, c:c + 4, :])
            self.wpanels[key] = t
        return self.wpanels[key]

    def _slab(self, g):
        """Whole-slab x^T tile [128, NDB, SG]; chunked DMAs (qSP)."""
        nc, cfg = self.nc, self.cfg
        if g not in self.slabs:
            t = self.xp.tile([128, cfg.NDB, cfg.SG], F16, tag="xs",
                             name=f"xs{g}")
            src = self.xT[:, g * cfg.SG:(g + 1) * cfg.SG].rearrange(
                "(db p) c -> p db c", p=128)
            if g == 0:
                # fine chunks so the first matmuls start after ~128KB
                for db in range(4):
                    nc.sync.dma_start(out=t[:, db, :], in_=src[:, db, :])
                for c in range(1, 4):
                    nc.sync.dma_start(out=t[:, 4 * c:4 * (c + 1), :],
                                      in_=src[:, 4 * c:4 * (c + 1), :])
            else:
                for c in range(4):
                    nc.sync.dma_start(out=t[:, 4 * c:4 * (c + 1), :],
                                      in_=src[:, 4 * c:4 * (c + 1), :])
            self.slabs[g] = t
            if not self.tables_loaded:
                self.tables_loaded = True
                nc.sync.dma_start(out=self.band_sb, in_=self.band[:, :])
                nc.sync.dma_start(out=self.cos_sb, in_=self.cosT[:, :])
                nc.sync.dma_start(out=self.sin_sb, in_=self.sinT[:, :])
        return self.slabs[g]

    def proj_segment(self, b, h):
        """Generator: projections (Q,K,V) of head h over batch b's slabs.

        Yields after each contraction step (NST_G matmuls) so the driver
        can interleave attention work.
        """
        nc, cfg = self.nc, self.cfg
        S, SG, ST, NDB = cfg.S, cfg.SG, cfg.ST, cfg.NDB
        for g in range(b * cfg.GPB, (b + 1) * cfg.GPB):
            xs = self._slab(g)
            for proj in range(3):
                w_t = self._wpanel(proj, h)
                accs = [self.psJ.tile([128, ST], F32, tag="pj",
                                      name=f"pj{st_i}")
                        for st_i in range(cfg.NST_G)]
                for db in range(NDB):
                    for st_i in range(cfg.NST_G):
                        nc.tensor.matmul(
                            accs[st_i],
                            lhsT=w_t[:, db, :],
                            rhs=xs[:, db, st_i * ST:(st_i + 1) * ST],
                            start=(db == 0), stop=(db == NDB - 1))
                    yield
                for st_i in range(cfg.NST_G):
                    gcol = g * SG + st_i * ST
                    scol = gcol % S
                    acc = accs[st_i]
                    if proj < 2:
                        dst = (self.QT if proj == 0 else
                               self.KT)[:, h, gcol:gcol + ST]
                        t1 = self.rp.tile([128, ST], F32, tag="t1")
                        nc.vector.tensor_mul(
                            t1, acc, self.cos_sb[:, scol:scol + ST])
                        t2 = self.rp.tile([128, ST], F32, tag="t2")
                        nc.vector.tensor_mul(
                            t2[0:64], acc[64:128],
                            self.sin_sb[0:64, scol:scol + ST])
                        nc.vector.tensor_mul(
                            t2[64:128], acc[0:64],
                            self.sin_sb[64:128, scol:scol + ST])
                        nc.vector.tensor_sub(dst[0:64], t1[0:64], t2[0:64])
                        nc.vector.tensor_add(dst[64:128], t1[64:128],
                                             t2[64:128])
                    else:
                        vstage = self.stp.tile([128, ST], F16, tag="vst")
                        nc.scalar.copy(out=vstage, in_=acc)
                        for jj in range(ST // 128):
                            gc = gcol + jj * 128
                            b_idx, kblk = gc // S, (gc % S) // 128
                            pst = self.psX.tile([128, 128], F16, tag="px",
                                                name="pst")
                            nc.tensor.transpose(
                                pst, vstage[:, jj * 128:(jj + 1) * 128],
                                self.ident)
                            nc.scalar.copy(
                                out=self.Vhat[:, h, b_idx, kblk, :],
                                in_=pst)
                    yield

    def attn_segment(self, b, h):
        """Generator: causal attention for (h, b).

        KPG-free pipeline: scores for k-block i+2 are emitted before the
        PV of block i (lookahead 2) so the exp latency hides under queued
        matmuls.  Diagonal blocks compute only columns [d, ST) with a
        fixed 128-wide triangle mask; the excluded pt region is zeroed
        off the critical chain.
        """
        nc, cfg = self.nc, self.cfg
        S, ST, RPC = cfg.S, cfg.ST, cfg.RPC
        LA = 2
        tri = self.band_sb
        for qt in range(S // ST):
            q0 = qt * ST
            nkb = (q0 + ST) // 128
            po = self.psO.tile([128, ST], F32, tag="po")
            pa = self.pap.tile([128, ST], F16, tag="pa")
            state = {}
            pa_state = {}

            def emit_sc(kb):
                d = kb * 128 - q0
                lo = max(d, 0)
                ps = self.psS.tile([128, ST], F32, tag="ps")
                pt = self.ptp.tile([128, ST], F16, tag="pt")
                nc.tensor.matmul(
                    ps[:, lo:ST],
                    lhsT=self.KT[:, h, b * S + kb * 128:
                                 b * S + kb * 128 + 128],
                    rhs=self.QT[:, h, b * S + q0 + lo:b * S + q0 + ST],
                    start=True, stop=True)
                if lo > 0:
                    nc.vector.memset(pt[:, 0:lo], 0.0)
                if d >= 0:
                    nc.vector.tensor_add(ps[:, lo:lo + 128],
                                         ps[:, lo:lo + 128], tri)
                nc.scalar.activation(pt[:, lo:ST], ps[:, lo:ST], EXP)
                if kb > 0:
                    flush_pa(kb - 1)
                state[kb] = (pt, lo)
                pa_state[kb] = (pt, lo)

            def flush_pa(kb):
                pt, lo = pa_state.pop(kb)
                if kb == 0:
                    nc.vector.tensor_copy(pa, pt)
                else:
                    nc.vector.tensor_add(pa[:, lo:ST], pa[:, lo:ST],
                                         pt[:, lo:ST])

            def emit_pv(kb):
                pt, lo = state.pop(kb)
                nc.tensor.matmul(
                    po[:, lo:ST] if lo > 0 else po,
                    lhsT=self.Vhat[:, h, b, kb, :],
                    rhs=pt[:, lo:ST],
                    start=(kb == 0), stop=(kb == nkb - 1))

            for i in range(nkb + LA):
                if i < nkb:
                    emit_sc(i)
                if i >= LA:
                    emit_pv(i - LA)
                if i % 2 == 1 or i >= nkb:
                    yield
            flush_pa(nkb - 1)
            pd = self.psD.tile([128, ST], F32, tag="pd")
            nc.tensor.matmul(pd, lhsT=self.ones_sb, rhs=pa,
                             start=True, stop=True)
            rec = self.rcp.tile([128, ST], F32, tag="rec")
            nc.vector.reciprocal_approx_fast(out=rec, in_=pd)
            aot = self.aotp.tile([128, ST], F16, tag="aot")
            nc.vector.tensor_mul(aot, po, rec)
            CSZ = min(ST, RPC)
            for ci in range(ST // CSZ):
                r0 = b * S + q0 + ci * CSZ
                nc.gpsimd.dma_start(
                    out=self.a2a_ins[h][r0 // RPC, :,
                                        r0 % RPC:r0 % RPC + CSZ],
                    in_=aot[:, ci * CSZ:(ci + 1) * CSZ])


def _outproj(nc, cfg, st, a2a_outs, aip, accp, obp, psU, out_rows):
    """out_rows[s', :] = AO^T_full[:, s'].T @ wo, contraction over heads.

    One PSUM round per (h, ss): round h accumulates that head's 8 blocks.
    The h=0 half of wo is already resident (preloaded during attention);
    the h=1 half streams in under the h=0 matmuls.  h=0 partial sums park
    in SBUF and are added during the h=1 pass.
    """
    D, HPC, RPC = cfg.D, cfg.HPC, cfg.RPC
    NH = cfg.NHB // HPC                   # blocks per head-group (8)
    NDC = D // 512
    NSS = RPC // 128

    # a2a_out tiles; h>0 loads emitted late (with that h's wo reload) so
    # the qSP FIFO order is [aip h0][wo h1][aip h1][out_rows]
    aot_sb = {}

    def load_aip(h):
        for j in range(NH):
            a = aip.tile([128, RPC], F16, tag=f"ai{h}_{j}",
                         name=f"ai_sb{h}_{j}")
            nc.sync.dma_start(out=a, in_=a2a_outs[h][j, :, :])
            aot_sb[(h, j)] = a

    load_aip(0)
    def wo_rhs(h, j, c0, c1):
        if h == 0:
            return st.WO[:, j, c0:c1]
        # h=1 wo blocks live in the dead QT/KT space: panels 0-3 in QT,
        # 4-7 in KT, flat-addressed [hh, 2048-col half]
        tile_ = st.QT if j < 4 else st.KT
        base = ((j % 4) % 2) * 2048
        return tile_[:, (j % 4) // 2, base + c0:base + c1]

    acc = {}
    for h in range(HPC):
        if h > 0:
            # emitted AFTER attention (QT/KT last reads) so the WAR frees
            # immediately; these loads hide under h=0's matmuls
            for j in range(NH):
                nc.sync.dma_start(out=wo_rhs(h, j, 0, D),
                                  in_=st.wo_dram[h, :, j, :])
            load_aip(h)
        for ss in range(NSS):
            pu = [psU.tile([128, 512], F32, tag="pu", name=f"pu_{dct}")
                  for dct in range(NDC)]
            for j in range(NH):
                for dct in range(NDC):
                    nc.tensor.matmul(
                        pu[dct],
                        lhsT=aot_sb[(h, j)][:, ss * 128:(ss + 1) * 128],
                        rhs=wo_rhs(h, j, dct * 512, (dct + 1) * 512),
                        start=(j == 0), stop=(j == NH - 1))
            for dct in range(NDC):
                if h == 0 and HPC > 1:
                    t = accp.tile([128, 512], F32, tag=f"acc{ss}_{dct}",
                                  name=f"acc{ss}_{dct}")
                    nc.vector.tensor_copy(t, pu[dct])
                    acc[(ss, dct)] = t
                else:
                    ob = obp.tile([128, 512], F32, tag="ob")
                    if HPC > 1:
                        nc.vector.tensor_add(ob, pu[dct], acc[(ss, dct)])
                    else:
                        nc.scalar.copy(out=ob, in_=pu[dct])
                    nc.sync.dma_start(
                        out=out_rows[ss * 128:(ss + 1) * 128,
                                     dct * 512:(dct + 1) * 512],
                        in_=ob)


# ---------------------------------------------------------------------------
# Host side
# ---------------------------------------------------------------------------

def _rope_perm(hd):
    return np.concatenate([np.arange(0, hd, 2), np.arange(1, hd, 2)])


def prepare_inputs(cfg: Cfg, x, freq_cis, wq_base, wk_base, wv_base, head_a,
                   head_b, q_a, q_b, k_a, k_b, v_a, v_b, wo):
    """Fold LoRA + softmax scale + RoPE permutation into per-core weights."""
    B, S, D, HD, HPC, NC_ = cfg.B, cfg.S, cfg.D, cfg.HD, cfg.HPC, cfg.NCORES
    HEADS = HPC * NC_
    NDB = cfg.NDB
    LORA_SCALE = 2.0
    sm = 1.0 / math.sqrt(HD)

    def fold(w_base, oa, ob):
        w = w_base.astype(np.float64).copy()
        only = LORA_SCALE * (oa.astype(np.float64) @ ob.astype(np.float64))
        hoff = LORA_SCALE * (head_a.astype(np.float64)
                             @ head_b.astype(np.float64))
        w += hoff
        w += np.tile(only, (1, HEADS))
        return w

    wq_eff = fold(wq_base, q_a, q_b) * sm
    wk_eff = fold(wk_base, k_a, k_b)
    wv_eff = fold(wv_base, v_a, v_b)

    perm = _rope_perm(HD)
    for h in range(HEADS):
        cols = h * HD + perm
        wq_eff[:, h * HD:(h + 1) * HD] = wq_eff[:, cols]
        wk_eff[:, h * HD:(h + 1) * HD] = wk_eff[:, cols]
    wq_eff = wq_eff.astype(np.float16)
    wk_eff = wk_eff.astype(np.float16)
    wv_eff = wv_eff.astype(np.float16)

    xT = np.ascontiguousarray(x.reshape(cfg.BS, D).T.astype(np.float16))

    cos = freq_cis[:S, :, 0].T.astype(np.float16)   # [64, S]
    sin = freq_cis[:S, :, 1].T.astype(np.float16)
    cosT = np.ascontiguousarray(np.concatenate([cos, cos], axis=0))
    sinT = np.ascontiguousarray(np.concatenate([sin, sin], axis=0))

    ii = np.arange(128)[:, None]
    cc = np.arange(128)[None, :]
    band = np.where(ii <= cc, 0.0, NEG).astype(np.float32)

    def panelize(w_core):
        # [D, HPC*128] -> [HPC, 128, NDB, 128] (p-major per head panel)
        out = np.empty((HPC, 128, NDB, 128), np.float16)
        for h in range(HPC):
            p = w_core[:, h * 128:(h + 1) * 128]          # [D, 128]
            out[h] = p.reshape(NDB, 128, 128).transpose(1, 0, 2)
        return np.ascontiguousarray(out)

    # wo: h-major block permutation, then p-major panels
    wo16 = wo.astype(np.float16)
    NH = cfg.NHB // HPC
    wo_p = np.empty((HPC, 128, NH, D), np.float16)
    for h in range(HPC):
        for j in range(NH):
            blk = wo16[(j * HPC + h) * 128:(j * HPC + h + 1) * 128, :]
            wo_p[h, :, j, :] = blk
    wo_p = np.ascontiguousarray(wo_p)

    in_maps = []
    for c in range(NC_):
        sl = slice(c * HPC * HD, (c + 1) * HPC * HD)
        in_maps.append(dict(
            xT=xT,
            wq=panelize(wq_eff[:, sl]),
            wk=panelize(wk_eff[:, sl]),
            wv=panelize(wv_eff[:, sl]),
            wo=wo_p,
            cosT=cosT, sinT=sinT, band=band,
        ))
    return in_maps


_BASS_CACHE = {}


def _get_bass(cfg: Cfg):
    key = (cfg.B, cfg.S, cfg.D, cfg.HPC, cfg.NCORES)
    if key not in _BASS_CACHE:
        _BASS_CACHE[key] = build_bass(cfg)
    return _BASS_CACHE[key]


def kernel(**inputs) -> np.ndarray:
    from concourse.bass_utils import run_bass_kernel_spmd

    x = np.asarray(inputs["x"])
    B, S, D = x.shape
    cfg = Cfg(B=B, S=S, D=D, HEADS=16, NCORES=8)
    in_maps = prepare_inputs(cfg, **{k: np.asarray(v)
                                     for k, v in inputs.items()})
    nc = _get_bass(cfg)
    res = run_bass_kernel_spmd(nc, in_maps, list(range(cfg.NCORES)))
    rows = np.concatenate([res.results[c]["out_rows"]
                           for c in range(cfg.NCORES)], axis=0)
    return rows.reshape(B, S, D).astype(np.float32)
